# revision 5
# baseline (speedup 1.0000x reference)
"""AWing loss kernel for Trainium2 (8 NeuronCores, pure data parallel).

Problem (hardcoded): prediction/target f32 [32, 68, 128, 128] -> scalar f32
    loss = mean(awing(pred, tgt) * mask),  mask = 1 + 10*[dilate3x3(tgt) > 0.2]

Branch-free math (exact):
    d   = |p - t|
    dc  = clamp(d, 1e-30, 0.5)
    e   = 2.1 - t
    EZ  = dc^e = exp(e*ln(dc))          # = d^e (d<.5) or 0.5^e (d>=.5)
    SP  = ln(1+EZ)                      # softplus branch-merge
    E2  = exp(-SP) = 1/(1+EZ)
    q2  = (1-E2)*(4.2-2t) = 2e*sigma
    loss/14 = SP + q2*relu(d-0.5)
    relu(d-0.5) = max(|x|,0.5)-0.5 in one tensor_scalar op
    mask m in {1,11}: m10 = 10*[conv3x3([t>0.2]) >= 0.5], m = m10+1
    result = 14/N * (sum(m*SP) + sum(m*q2*relu(d-0.5)))

This toolchain's walrus encodes at most ONE sync wait per instruction;
Tile emits more. _fission_multiwaits() splits surplus waits onto NoOps
inserted before the offending instruction on the same engine.

Sharding: batch dim 32 -> 4 batches (272 (b,c) planes) per core.
Layout: SBUF tile [128(h), 16(plane), 128(w)]: 3x3 dilation = free-dim
row-sums (DVE) + tridiagonal-ones matmul over partitions (PE).
"""

import numpy as np
from contextlib import ExitStack

B, C, H, W = 32, 68, 128, 128
NCORES = 8
PPC = (B // NCORES) * C          # 272 planes per core
NP = 16                          # planes per SBUF tile
NT = PPC // NP                   # 17 tiles per core
F = NP * W                       # 2048 free elements per partition per tile
N_TOTAL = B * C * H * W

_CACHE = {}


def _build_nc(repeat=1, loop_reps=0):
    import concourse.bass as bass
    import concourse.mybir as mybir
    import ml_dtypes
    from concourse.tile import TileContext

    f32 = mybir.dt.float32
    bf16 = mybir.dt.bfloat16
    Alu = mybir.AluOpType
    Act = mybir.ActivationFunctionType

    nc = bass.Bass(num_swdge_queues=1)
    # Host pre-transposes to [H, PPC, 2, W]: every SBUF partition (h) then
    # reads one contiguous 16 KB chunk per tile (128 big descriptors at
    # line rate) instead of 32 strided 512 B chunks (descriptor-bound).
    pt_d = nc.dram_tensor("pt", [H, PPC, 2, W], f32, kind="ExternalInput")
    out_d = nc.dram_tensor("out", [128, 1], f32, kind="ExternalOutput")

    # Tridiagonal-ones [128,128]: (tri @ x)[h] = x[h-1]+x[h]+x[h+1] (SAME).
    tri_np = np.zeros((H, H), dtype=ml_dtypes.bfloat16)
    for i in range(H):
        for j2 in range(max(0, i - 1), min(H, i + 2)):
            tri_np[i, j2] = 1.0
    tri_d = nc.inline_tensor(tri_np, name="tri")

    # const AP for the Sign bias (-0.5), same pattern Bass uses internally
    _c = nc.alloc_sbuf_tensor("const-f32-m0p5", [128, 1], f32)
    nc.gpsimd.memset(_c.ap(), -0.5)
    nc.const_aps.aps[(f32, -0.5)] = _c.ap()
    nc.all_engine_barrier()

    with TileContext(nc) as tc, ExitStack() as ctx:
        cpool = ctx.enter_context(tc.tile_pool(name="cpool", bufs=1))
        io = ctx.enter_context(tc.tile_pool(name="io", bufs=2))
        wk2 = ctx.enter_context(tc.tile_pool(name="wk2", bufs=2))
        psp = ctx.enter_context(tc.tile_pool(name="psp", bufs=2, space="PSUM"))

        tri_s = cpool.tile([H, H], bf16, name="tri_s")
        nc.sync.dma_start(tri_s[:], tri_d[:, :])
        acc1 = cpool.tile([128, NT], f32, name="acc1")
        acc2 = cpool.tile([128, NT], f32, name="acc2")

        import contextlib
        loop_cm = tc.For_i(0, loop_reps, 1) if loop_reps else contextlib.nullcontext()
        with loop_cm:
            for j in [jj for _ in range(repeat) for jj in range(NT)]:
                # one DMA per tile: [128(h), NP, 2(p/t), W], straight slice of
                # the host-transposed layout -> 16 KB contiguous per partition
                pts = io.tile([128, NP, 2, W], f32, name="pts", tag="pts")
                nc.sync.dma_start(
                    pts[:], pt_d[:, j * NP:(j + 1) * NP, :, :])
                ptv = pts[:, :, 0, :]
                ttv = pts[:, :, 1, :]

                # x = p - t       (waits: DMA only)
                x = wk2.tile([128, NP, W], f32, name="x", tag="x", bufs=1)
                nc.vector.tensor_tensor(x[:], ptv, ttv, Alu.subtract)
                # d = |x|  (ACT; Abs is resident in every table set)
                dab = wk2.tile([128, NP, W], f32, name="dab", tag="dab")
                nc.scalar.activation(dab[:], x[:], Act.Abs)
                # rdmr = relu(d-0.5) = max(d,0.5) - 0.5   (GPSIMD, bf16 out)
                rdmr = wk2.tile([128, NP, W], bf16, name="rdmr", tag="rdmr")
                nc.gpsimd.tensor_scalar(rdmr[:], dab[:], 0.5, -0.5, Alu.max, Alu.add)
                # ind = [t > 0.2] as bf16   (GPSIMD)
                ind = wk2.tile([128, NP, W], bf16, name="ind", tag="ind")
                nc.gpsimd.tensor_scalar(ind[:], ttv, 0.2, None, Alu.is_gt)
                # em = 4.2 - 2t = 2e   (GPSIMD, bf16 out)
                em = wk2.tile([128, NP, W], bf16, name="em", tag="em")
                nc.gpsimd.tensor_scalar(em[:], ttv, -2.0, 4.2, Alu.mult, Alu.add)

                # L = ln(d)   (ACT; ln(0) -> -inf propagates correctly)
                L = wk2.tile([128, NP, W], f32, name="L", tag="L")
                nc.scalar.activation(L[:], dab[:], Act.Ln)
                # Lc = min(L, -ln2)  ==  ln(min(d, 0.5))
                Lc = wk2.tile([128, NP, W], f32, name="Lc", tag="Lc", bufs=1)
                nc.vector.tensor_scalar(Lc[:], L[:], -0.6931471805599453, None, Alu.min)
                # zn = (t - 2.1) * Lc = -e*ln(dc)   (DVE waits ACT)
                zn = wk2.tile([128, NP, W], f32, name="zn", tag="zn")
                nc.vector.scalar_tensor_tensor(
                    zn[:], ttv, 2.1, Lc[:], Alu.subtract, Alu.mult)
                # ez = exp(-zn) = dc^e
                ez = wk2.tile([128, NP, W], f32, name="ez", tag="ez")
                nc.scalar.activation(ez[:], zn[:], Act.Exp, scale=-1.0)
                # sp = ln(1 + ez)
                sp = wk2.tile([128, NP, W], f32, name="sp", tag="sp")
                nc.scalar.activation(sp[:], ez[:], Act.Ln, bias=1.0)
                # e2 = exp(-sp) = 1/(1+ez)  (bf16 out: feeds bf16 R-branch)
                e2 = wk2.tile([128, NP, W], bf16, name="e2", tag="e2")
                nc.scalar.activation(e2[:], sp[:], Act.Exp, scale=-1.0)

                # 3-tap row sums of ind (SAME edges), bf16
                rs2 = wk2.tile([128, NP, W], bf16, name="rs2", tag="rs2", bufs=1)
                nc.vector.tensor_tensor(
                    rs2[:, :, 0:W - 1], ind[:, :, 0:W - 1], ind[:, :, 1:W], Alu.add)
                nc.vector.tensor_copy(rs2[:, :, W - 1:W], ind[:, :, W - 1:W])
                rs3 = wk2.tile([128, NP, W], bf16, name="rs3", tag="rs3", bufs=1)
                nc.vector.tensor_tensor(
                    rs3[:, :, 1:W], rs2[:, :, 0:W - 1], rs2[:, :, 1:W], Alu.add)
                nc.vector.tensor_copy(rs3[:, :, 0:1], rs2[:, :, 0:1])
                # column 3-tap via tridiagonal matmul -> PSUM (f32 counts 0..9)
                cs = psp.tile([128, F], f32, name="cs", tag="cs")
                rs3f = rs3[:].rearrange("h a b -> h (a b)")
                for k in range(F // 512):
                    nc.tensor.matmul(
                        cs[:, k * 512:(k + 1) * 512], tri_s[:],
                        rs3f[:, k * 512:(k + 1) * 512], start=True, stop=True)
                # s = sign(cs-0.5) in {-1,1}   (ACT reads PSUM)
                sg = wk2.tile([128, NP, W], bf16, name="sg", tag="sg")
                csv = cs[:].rearrange("h (a b) -> h a b", a=NP)
                nc.scalar.activation(sg[:], csv, Act.Sign, bias=-0.5)
                # mt = 5s+6 in {1,11}   (DVE bf16 TS: 4x mode)
                mt = wk2.tile([128, NP, W], bf16, name="mt", tag="mt")
                nc.vector.tensor_scalar(mt[:], sg[:], 5.0, 6.0, Alu.mult, Alu.add)

                # rm = m * relu(|x|-1/2)        (bf16 TT: 2x mode)
                rm = wk2.tile([128, NP, W], bf16, name="rm", tag="rm")
                nc.vector.tensor_tensor(rm[:], mt[:], rdmr[:], Alu.mult)
                # g2 = 2e * rm                  (bf16 TT: 2x mode)
                g2 = wk2.tile([128, NP, W], bf16, name="g2", tag="g2")
                nc.vector.tensor_tensor(g2[:], em[:], rm[:], Alu.mult)
                # acc2[:, j] = sum((e2-1) * g2) = -sum(m * q2 * relu(d-1/2))
                dump1 = wk2.tile([128, NP, W], bf16, name="dump1", tag="dumpb", bufs=1)
                nc.vector.scalar_tensor_tensor(
                    dump1[:], e2[:], 1.0, g2[:], Alu.subtract, Alu.mult,
                    accum_out=acc2[:, j:j + 1])
                # acc1[:, j] = sum((m10+1) * sp) = sum(m * SP)
                dump2 = wk2.tile([128, NP, W], bf16, name="dump2", tag="dumpb", bufs=1)
                nc.vector.scalar_tensor_tensor(
                    dump2[:], mt[:], 1.0, sp[:], Alu.mult, Alu.mult,
                    accum_out=acc1[:, j:j + 1])


        tot = cpool.tile([128, NT], f32, name="tot")
        nc.vector.tensor_tensor(tot[:], acc1[:], acc2[:], Alu.subtract)
        vec = cpool.tile([128, 1], f32, name="vec")
        nc.vector.tensor_reduce(
            vec[:], tot[:], axis=mybir.AxisListType.X, op=Alu.add)
        nc.sync.dma_start(out_d[:, :], vec[:])

    _fission_multiwaits(nc, mybir)
    return nc


def _fission_multiwaits(nc, mybir):
    """walrus here encodes at most ONE sync wait per instruction; Tile emits
    more. Split: surplus waits move to NoOps inserted just before the
    instruction on the same engine (program order preserves semantics)."""
    nid = [0]

    def mk_nop(engine, wait):
        nid[0] += 1
        nop = mybir.InstNoOp(name=f"WF-{nid[0]}", ins=[], outs=[])
        nop.engine = engine
        nop.sync_info = mybir.SyncInfo(on_wait=[wait], on_update=[])
        return nop

    for f in nc.m.functions:
        for bb in f.blocks:
            out = []
            for ins in bb.instructions:
                si = getattr(ins, "sync_info", None)
                if si is not None and len(si.on_wait) > 1:
                    waits = list(si.on_wait)
                    for w in waits[:-1]:
                        out.append(mk_nop(ins.engine, w))
                    ins.sync_info = mybir.SyncInfo(
                        on_wait=[waits[-1]], on_update=list(si.on_update))
                out.append(ins)
            bb.instructions[:] = out


def _get_nc():
    if "nc" not in _CACHE:
        _CACHE["nc"] = _build_nc()
    return _CACHE["nc"]


def prep_inmaps(prediction, target):
    p = np.asarray(prediction, dtype=np.float32).reshape(NCORES, PPC, H, W)
    t = np.asarray(target, dtype=np.float32).reshape(NCORES, PPC, H, W)
    stacked = np.stack([p, t], axis=2)  # [NCORES, PPC, 2, H, W]
    # host-side transpose to [NCORES, H, PPC, 2, W] so the device DMA is a
    # plain affine slice with 16 KB contiguous per partition (see _build_nc)
    arr = np.ascontiguousarray(stacked.transpose(0, 3, 1, 2, 4))
    return [{"pt": arr[c]} for c in range(NCORES)]


def run(prediction, target, trace=False, **trace_kw):
    from concourse.bass_utils import run_bass_kernel_spmd

    nc = _get_nc()
    in_maps = prep_inmaps(prediction, target)
    res = run_bass_kernel_spmd(
        nc, in_maps, core_ids=list(range(NCORES)), trace=trace, **trace_kw)
    total = 0.0
    for r in res.results:
        total += np.asarray(r["out"], dtype=np.float64).sum()
    value = np.float32(14.0 * total / N_TOTAL)
    return value, res


def kernel(prediction, target):
    value, _ = run(prediction, target)
    return value

